# revision 16
# baseline (speedup 1.0000x reference)
"""GridMask kernel for Trainium2, 8-core data parallel.

out[b,h,w,c] = x[b,h,w,c] * row_keep[b,h] * col_keep[b,w]

The grid mask is separable and zeroes ~50% of rows outright, so half the
input never influences the output and half the output is all-zero rows.
The kernel exploits both sides:

- Read side: gpsimd indirect DMAs (SWDGE gather) load ONLY the 2-row
  half-blocks containing at least one kept image row, via index tables
  computed host-side with exact integer math. Fully-dead half-blocks
  carry a sentinel index that fails the DGE bounds check and is skipped:
  no descriptor, no HBM traffic. Probed HW semantics: one descriptor per
  partition of the offset AP, start row = the partition's first index,
  length = the dst partition's free size; an invalid index leaves the
  partition untouched.
- Write side: the Bass runtime hands every ExternalOutput to the kernel
  pre-zeroed (native path np.zeros()es output buffers; the PJRT path
  donates zero buffers - kernels that don't write every element rely on
  this contract). So all-zero half-blocks of y are never written: an
  indirect SCATTER with matching index tables writes back only the live
  half-blocks; sentinel partitions leave the pre-zeroed rows alone.

y is split into one DRAM tensor per image: the Tile framework treats
scatters into a shared tensor as write-after-write hazards and
serializes them on the single SWDGE ring; disjoint tensors let the ring
stream continuously (per-image scatter halves still serialize against
each other, which matches their data dependencies anyway).

Stale SBUF data in dead half-blocks never reaches y (never scattered),
so no buffer initialization is needed. Dead rows inside live half-blocks
are zeroed by the row-mask STT, exactly the reference's 0*x.

Per core: 4 images, SBUF tile per image [128, 6144], partition p =
image rows 4p..4p+3; half a = rows 4p..4p+1, half b = rows 4p+2..4p+3
(12 KB descriptors). Column mask (bf16 0/1, exact) broadcasts to
[128,1536] f32 PSUM via K=1 matmuls; row mask enters the STT as a
per-partition scalar. Total HBM traffic ~6.5 MB in + ~6.5 MB out per
core vs the dense kernel's 25.2 MB.
"""

import math

import numpy as np

import concourse.mybir as mybir
from concourse import bacc, bass, tile
from concourse.bass_utils import run_bass_kernel_spmd

B, H, W, C = 32, 512, 512, 3
D1 = 96
HH = math.ceil(math.sqrt(H * H + W * W))  # 725
OFF_H = (HH - H) // 2  # 106
OFF_W = (HH - W) // 2  # 106

NCORES = 8
BPC = B // NCORES  # images per core
FREE = W * C  # 1536 floats per image row

F32 = mybir.dt.float32
BF16 = mybir.dt.bfloat16
I32 = mybir.dt.int32

_CACHE: dict = {}


def _build_masks(d_raw, st_h_raw, st_w_raw):
    """Exact replica of the reference's integer mask math, in numpy."""
    d = D1 + d_raw.astype(np.int64)  # [B] stripe period
    l = (d + 1) // 2  # ceil(d * 0.5) for integer d
    st_h = st_h_raw.astype(np.int64) % d
    st_w = st_w_raw.astype(np.int64) % d
    yy = OFF_H + np.arange(H, dtype=np.int64)
    xx = OFF_W + np.arange(W, dtype=np.int64)
    row_zero = ((yy[None, :] - st_h[:, None]) % d[:, None]) < l[:, None]
    col_zero = ((xx[None, :] - st_w[:, None]) % d[:, None]) < l[:, None]
    row_keep = (~row_zero).astype(np.float32)  # [B,H]
    col_keep = (~col_zero).astype(np.float32)  # [B,W]
    return row_keep, col_keep


NTILES = BPC  # one image per tile
RPP = H // 128  # 4 consecutive image rows per partition
TILE_FREE = RPP * FREE  # 6144 floats = 24 KB per partition
HALF_FREE = TILE_FREE // 2  # 2-row half-block
NROWS = NTILES * H  # global row count per core (2048)
SENTINEL = 3000  # > any valid row id: fails bounds check, half skipped


def _build_nc():
    nc = bacc.Bacc(None)
    # x as a flat row table [2048, 1536]: global row g = t*512 + r. The
    # gathers index axis 0 (coef = 1536 floats); each valid index fetches
    # a 2-row half-block (dst partition free size 3072).
    x = nc.dram_tensor("x", [NROWS, FREE], F32, kind="ExternalInput")
    rowm = nc.dram_tensor("rowm", [128, NTILES * RPP], F32, kind="ExternalInput")
    # Gather tables (global row ids): idxg[p, 2t+h] = t*512 + 4p + 2h if
    # image t's rows 4p+2h..4p+2h+1 contain a kept row, else SENTINEL.
    idxg = nc.dram_tensor("idxg", [128, NTILES * 2], I32, kind="ExternalInput")
    # Scatter tables (in-image row ids): idxs[p, 2t+h] = 4p + 2h or SENTINEL.
    idxs = nc.dram_tensor("idxs", [128, NTILES * 2], I32, kind="ExternalInput")
    colm = nc.dram_tensor("colm", [1, NTILES * FREE], BF16, kind="ExternalInput")
    ys = [
        nc.dram_tensor(f"y{t}", [H, FREE], F32, kind="ExternalOutput")
        for t in range(NTILES)
    ]

    mult = mybir.AluOpType.mult
    with tile.TileContext(nc) as tc:
        with (
            tc.tile_pool(name="const", bufs=1) as cpool,
            tc.tile_pool(name="io", bufs=NTILES) as iop,
            tc.tile_pool(name="psum", bufs=2, space="PSUM") as psp,
        ):
            idxg_sb = cpool.tile([128, NTILES * 2], I32, tag="idxg")
            nc.gpsimd.dma_start(idxg_sb[:], idxg[:])
            idxs_sb = cpool.tile([128, NTILES * 2], I32, tag="idxs")
            nc.gpsimd.dma_start(idxs_sb[:], idxs[:])
            rowm_sb = cpool.tile([128, NTILES * RPP], F32, tag="rowm")
            nc.gpsimd.dma_start(rowm_sb[:], rowm[:])
            colm_sb = cpool.tile([1, NTILES * FREE], BF16, tag="colm")
            nc.gpsimd.dma_start(colm_sb[:], colm[:])
            ones_sb = cpool.tile([1, 128], BF16, tag="ones")
            nc.vector.memset(ones_sb[:], 1.0)

            # Hybrid reads. The SWDGE ring's ~253 GB/s descriptor-dispatch
            # cadence is the kernel's wall while the sync/scalar HWDGE
            # queues (330+ GB/s each) sit idle, so images 0 and 1 load
            # DENSELY on those queues (extra bytes, but off the ring, and
            # a dense tile has no stale slots) while images 2 and 3 use
            # ring gathers that skip dead half-blocks. This balances ring
            # work (~9.8 MB) against total DMA-engine work (~16 MB).
            # Gathers go first so they stream ahead of the data-dependent
            # scatters; halves let the first mask-multiply start as soon
            # as half of an image has landed.
            tiles = []
            for t in range(NTILES):
                xt = iop.tile([128, TILE_FREE], F32, tag="xt", name=f"xt{t}")
                if t < 2:
                    eng = nc.scalar if t == 0 else nc.sync
                    eng.dma_start(xt[:], x[t * H : (t + 1) * H, :])
                else:
                    for h in range(2):
                        nc.gpsimd.indirect_dma_start(
                            out=xt[:, h * HALF_FREE : (h + 1) * HALF_FREE],
                            out_offset=None,
                            in_=x[:],
                            in_offset=bass.IndirectOffsetOnAxis(
                                ap=idxg_sb[:, 2 * t + h : 2 * t + h + 1],
                                axis=0,
                            ),
                            bounds_check=NROWS - 1,
                            oob_is_err=False,
                        )
                tiles.append(xt)

            for t in range(NTILES):
                xt = tiles[t]
                cmask = psp.tile([128, FREE], F32, tag="cmask", name=f"cm{t}")
                for ch in range(FREE // 512):
                    sl = slice(t * FREE + ch * 512, t * FREE + (ch + 1) * 512)
                    nc.tensor.matmul(
                        cmask[:, ch * 512 : (ch + 1) * 512],
                        ones_sb[:],
                        colm_sb[:, sl],
                        start=True,
                        stop=True,
                    )
                for h in range(2):
                    for r in (2 * h, 2 * h + 1):
                        rs = slice(r * FREE, (r + 1) * FREE)
                        nc.vector.scalar_tensor_tensor(
                            xt[:, rs],
                            xt[:, rs],
                            rowm_sb[:, t * RPP + r : t * RPP + r + 1],
                            cmask[:],
                            op0=mult,
                            op1=mult,
                        )
                    # Scatter this half's live 2-row blocks; sentinel
                    # partitions leave the pre-zeroed y rows in place.
                    nc.gpsimd.indirect_dma_start(
                        out=ys[t][:],
                        out_offset=bass.IndirectOffsetOnAxis(
                            ap=idxs_sb[:, 2 * t + h : 2 * t + h + 1],
                            axis=0,
                        ),
                        in_=xt[:, h * HALF_FREE : (h + 1) * HALF_FREE],
                        in_offset=None,
                        bounds_check=H - 1,
                        oob_is_err=False,
                    )
    nc.compile()
    return nc


def _prep_inputs(x, d_raw, st_h_raw, st_w_raw):
    x = np.ascontiguousarray(np.asarray(x, dtype=np.float32))
    row_keep, col_keep = _build_masks(
        np.asarray(d_raw), np.asarray(st_h_raw), np.asarray(st_w_raw)
    )
    col_exp = np.repeat(col_keep, C, axis=1)  # [B, W*C]
    in_maps = []
    for c in range(NCORES):
        sl = slice(c * BPC, (c + 1) * BPC)
        xc = x[sl].reshape(NROWS, FREE)
        rk = row_keep[sl].astype(bool)  # [NTILES, H]
        # Half-block granularity: move rows 4p+2h..4p+2h+1 iff either is
        # kept (dead rows in a live half are zeroed by the rowm STT).
        hlive = rk.reshape(NTILES, 128, 2, 2).any(axis=3)  # [NTILES, 128, 2]
        local = (np.arange(128, dtype=np.int32) * RPP)[None, :, None] + (
            np.arange(2, dtype=np.int32) * 2
        )[None, None, :]
        glob = local + (np.arange(NTILES, dtype=np.int32) * H)[:, None, None]
        idxg = np.where(hlive, glob, SENTINEL).astype(np.int32)
        idxs = np.where(hlive, local, SENTINEL).astype(np.int32)
        # [NTILES, 128, 2] -> [128, NTILES*2]
        idxg = np.ascontiguousarray(
            idxg.transpose(1, 0, 2).reshape(128, NTILES * 2)
        )
        idxs = np.ascontiguousarray(
            idxs.transpose(1, 0, 2).reshape(128, NTILES * 2)
        )
        rm = np.ascontiguousarray(
            row_keep[sl]
            .reshape(NTILES, 128, RPP)
            .transpose(1, 0, 2)
            .reshape(128, NTILES * RPP)
        )
        cm = np.ascontiguousarray(col_exp[sl].reshape(1, NTILES * FREE)).astype(
            mybir.dt.np(BF16)
        )
        in_maps.append({"x": xc, "rowm": rm, "idxg": idxg, "idxs": idxs, "colm": cm})
    return in_maps


def kernel(x, d_raw, st_h_raw, st_w_raw):
    if "nc" not in _CACHE:
        _CACHE["nc"] = _build_nc()
    nc = _CACHE["nc"]
    in_maps = _prep_inputs(x, d_raw, st_h_raw, st_w_raw)
    res = run_bass_kernel_spmd(nc, in_maps, list(range(NCORES)))
    out = np.concatenate(
        [
            np.stack(
                [np.asarray(r[f"y{t}"]).reshape(H, W, C) for t in range(NTILES)]
            )
            for r in res.results
        ],
        axis=0,
    )
    return out


# revision 18
# speedup vs baseline: 1.0347x; 1.0347x over previous
"""GridMask kernel for Trainium2, 8-core data parallel.

out[b,h,w,c] = x[b,h,w,c] * row_keep[b,h] * col_keep[b,w]

The grid mask is separable and zeroes ~50% of rows outright, so half the
input never influences the output and half the output is all-zero rows.
The kernel exploits both sides:

- Read side: gpsimd indirect DMAs (SWDGE gather) load ONLY the 2-row
  half-blocks containing at least one kept image row, via index tables
  computed host-side with exact integer math. Fully-dead half-blocks
  carry a sentinel index that fails the DGE bounds check and is skipped:
  no descriptor, no HBM traffic. Probed HW semantics: one descriptor per
  partition of the offset AP, start row = the partition's first index,
  length = the dst partition's free size; an invalid index leaves the
  partition untouched.
- Write side: the Bass runtime hands every ExternalOutput to the kernel
  pre-zeroed (native path np.zeros()es output buffers; the PJRT path
  donates zero buffers - kernels that don't write every element rely on
  this contract). So all-zero half-blocks of y are never written: an
  indirect SCATTER with matching index tables writes back only the live
  half-blocks; sentinel partitions leave the pre-zeroed rows alone.

y is split into one DRAM tensor per image: the Tile framework treats
scatters into a shared tensor as write-after-write hazards and
serializes them on the single SWDGE ring; disjoint tensors let the ring
stream continuously (per-image scatter halves still serialize against
each other, which matches their data dependencies anyway).

Stale SBUF data in dead half-blocks never reaches y (never scattered),
so no buffer initialization is needed. Dead rows inside live half-blocks
are zeroed by the row-mask STT, exactly the reference's 0*x.

Per core: 4 images, SBUF tile per image [128, 6144], partition p =
image rows 4p..4p+3; half a = rows 4p..4p+1, half b = rows 4p+2..4p+3
(12 KB descriptors). Column mask (bf16 0/1, exact) broadcasts to
[128,1536] f32 PSUM via K=1 matmuls; row mask enters the STT as a
per-partition scalar. Total HBM traffic ~6.5 MB in + ~6.5 MB out per
core vs the dense kernel's 25.2 MB.
"""

import math

import numpy as np

import concourse.mybir as mybir
from concourse import bacc, bass, tile
from concourse.bass_utils import run_bass_kernel_spmd

B, H, W, C = 32, 512, 512, 3
D1 = 96
HH = math.ceil(math.sqrt(H * H + W * W))  # 725
OFF_H = (HH - H) // 2  # 106
OFF_W = (HH - W) // 2  # 106

NCORES = 8
BPC = B // NCORES  # images per core
FREE = W * C  # 1536 floats per image row

F32 = mybir.dt.float32
BF16 = mybir.dt.bfloat16
I32 = mybir.dt.int32

_CACHE: dict = {}


def _build_masks(d_raw, st_h_raw, st_w_raw):
    """Exact replica of the reference's integer mask math, in numpy."""
    d = D1 + d_raw.astype(np.int64)  # [B] stripe period
    l = (d + 1) // 2  # ceil(d * 0.5) for integer d
    st_h = st_h_raw.astype(np.int64) % d
    st_w = st_w_raw.astype(np.int64) % d
    yy = OFF_H + np.arange(H, dtype=np.int64)
    xx = OFF_W + np.arange(W, dtype=np.int64)
    row_zero = ((yy[None, :] - st_h[:, None]) % d[:, None]) < l[:, None]
    col_zero = ((xx[None, :] - st_w[:, None]) % d[:, None]) < l[:, None]
    row_keep = (~row_zero).astype(np.float32)  # [B,H]
    col_keep = (~col_zero).astype(np.float32)  # [B,W]
    return row_keep, col_keep


NTILES = BPC  # one image per tile
RPP = H // 128  # 4 consecutive image rows per partition
TILE_FREE = RPP * FREE  # 6144 floats = 24 KB per partition
HALF_FREE = TILE_FREE // 2  # 2-row half-block
NROWS = NTILES * H  # global row count per core (2048)
SENTINEL = 3000  # > any valid row id: fails bounds check, half skipped


def _build_nc():
    nc = bacc.Bacc(None)
    # x as a flat row table [2048, 1536]: global row g = t*512 + r. The
    # gathers index axis 0 (coef = 1536 floats); each valid index fetches
    # a 2-row half-block (dst partition free size 3072).
    x = nc.dram_tensor("x", [NROWS, FREE], F32, kind="ExternalInput")
    rowm = nc.dram_tensor("rowm", [128, NTILES * RPP], F32, kind="ExternalInput")
    # Gather tables (global row ids): idxg[p, 2t+h] = t*512 + 4p + 2h if
    # image t's rows 4p+2h..4p+2h+1 contain a kept row, else SENTINEL.
    idxg = nc.dram_tensor("idxg", [128, NTILES * 2], I32, kind="ExternalInput")
    # Scatter tables (in-image row ids): idxs[p, 2t+h] = 4p + 2h or SENTINEL.
    idxs = nc.dram_tensor("idxs", [128, NTILES * 2], I32, kind="ExternalInput")
    colm = nc.dram_tensor("colm", [1, NTILES * FREE], BF16, kind="ExternalInput")
    ys = [
        nc.dram_tensor(f"y{t}", [H, FREE], F32, kind="ExternalOutput")
        for t in range(NTILES)
    ]

    mult = mybir.AluOpType.mult
    with tile.TileContext(nc) as tc:
        with (
            tc.tile_pool(name="const", bufs=1) as cpool,
            tc.tile_pool(name="io", bufs=NTILES) as iop,
            tc.tile_pool(name="psum", bufs=2, space="PSUM") as psp,
        ):
            # Const loads ride the otherwise-idle sync HW queue so the
            # SWDGE ring's entry budget goes entirely to image traffic
            # and the first gather issues as soon as idxg lands.
            idxg_sb = cpool.tile([128, NTILES * 2], I32, tag="idxg")
            nc.sync.dma_start(idxg_sb[:], idxg[:])
            idxs_sb = cpool.tile([128, NTILES * 2], I32, tag="idxs")
            nc.sync.dma_start(idxs_sb[:], idxs[:])
            rowm_sb = cpool.tile([128, NTILES * RPP], F32, tag="rowm")
            nc.sync.dma_start(rowm_sb[:], rowm[:])
            colm_sb = cpool.tile([1, NTILES * FREE], BF16, tag="colm")
            nc.sync.dma_start(colm_sb[:], colm[:])
            ones_sb = cpool.tile([1, 128], BF16, tag="ones")
            nc.vector.memset(ones_sb[:], 1.0)

            # All gathers first: the SWDGE ring executes in order, so the
            # image loads stream back-to-back ahead of the data-dependent
            # scatters. Halves let the first mask-multiply start as soon
            # as half of image 0 has landed. (Dense loads on the static
            # queues were tried and regressed: they contend with the ring
            # for the 16 shared DMA engines exactly when image 0 must
            # arrive, pushing the whole mask-multiply chain out.)
            tiles = []
            for t in range(NTILES):
                xt = iop.tile([128, TILE_FREE], F32, tag="xt", name=f"xt{t}")
                for h in range(2):
                    nc.gpsimd.indirect_dma_start(
                        out=xt[:, h * HALF_FREE : (h + 1) * HALF_FREE],
                        out_offset=None,
                        in_=x[:],
                        in_offset=bass.IndirectOffsetOnAxis(
                            ap=idxg_sb[:, 2 * t + h : 2 * t + h + 1],
                            axis=0,
                        ),
                        bounds_check=NROWS - 1,
                        oob_is_err=False,
                    )
                tiles.append(xt)

            for t in range(NTILES):
                xt = tiles[t]
                cmask = psp.tile([128, FREE], F32, tag="cmask", name=f"cm{t}")
                for ch in range(FREE // 512):
                    sl = slice(t * FREE + ch * 512, t * FREE + (ch + 1) * 512)
                    nc.tensor.matmul(
                        cmask[:, ch * 512 : (ch + 1) * 512],
                        ones_sb[:],
                        colm_sb[:, sl],
                        start=True,
                        stop=True,
                    )
                for h in range(2):
                    for r in (2 * h, 2 * h + 1):
                        rs = slice(r * FREE, (r + 1) * FREE)
                        nc.vector.scalar_tensor_tensor(
                            xt[:, rs],
                            xt[:, rs],
                            rowm_sb[:, t * RPP + r : t * RPP + r + 1],
                            cmask[:],
                            op0=mult,
                            op1=mult,
                        )
                    # Scatter this half's live 2-row blocks; sentinel
                    # partitions leave the pre-zeroed y rows in place.
                    nc.gpsimd.indirect_dma_start(
                        out=ys[t][:],
                        out_offset=bass.IndirectOffsetOnAxis(
                            ap=idxs_sb[:, 2 * t + h : 2 * t + h + 1],
                            axis=0,
                        ),
                        in_=xt[:, h * HALF_FREE : (h + 1) * HALF_FREE],
                        in_offset=None,
                        bounds_check=H - 1,
                        oob_is_err=False,
                    )
    nc.compile()
    return nc


def _prep_inputs(x, d_raw, st_h_raw, st_w_raw):
    x = np.ascontiguousarray(np.asarray(x, dtype=np.float32))
    row_keep, col_keep = _build_masks(
        np.asarray(d_raw), np.asarray(st_h_raw), np.asarray(st_w_raw)
    )
    col_exp = np.repeat(col_keep, C, axis=1)  # [B, W*C]
    in_maps = []
    for c in range(NCORES):
        sl = slice(c * BPC, (c + 1) * BPC)
        xc = x[sl].reshape(NROWS, FREE)
        rk = row_keep[sl].astype(bool)  # [NTILES, H]
        # Half-block granularity: move rows 4p+2h..4p+2h+1 iff either is
        # kept (dead rows in a live half are zeroed by the rowm STT).
        hlive = rk.reshape(NTILES, 128, 2, 2).any(axis=3)  # [NTILES, 128, 2]
        local = (np.arange(128, dtype=np.int32) * RPP)[None, :, None] + (
            np.arange(2, dtype=np.int32) * 2
        )[None, None, :]
        glob = local + (np.arange(NTILES, dtype=np.int32) * H)[:, None, None]
        idxg = np.where(hlive, glob, SENTINEL).astype(np.int32)
        idxs = np.where(hlive, local, SENTINEL).astype(np.int32)
        # [NTILES, 128, 2] -> [128, NTILES*2]
        idxg = np.ascontiguousarray(
            idxg.transpose(1, 0, 2).reshape(128, NTILES * 2)
        )
        idxs = np.ascontiguousarray(
            idxs.transpose(1, 0, 2).reshape(128, NTILES * 2)
        )
        rm = np.ascontiguousarray(
            row_keep[sl]
            .reshape(NTILES, 128, RPP)
            .transpose(1, 0, 2)
            .reshape(128, NTILES * RPP)
        )
        cm = np.ascontiguousarray(col_exp[sl].reshape(1, NTILES * FREE)).astype(
            mybir.dt.np(BF16)
        )
        in_maps.append({"x": xc, "rowm": rm, "idxg": idxg, "idxs": idxs, "colm": cm})
    return in_maps


def kernel(x, d_raw, st_h_raw, st_w_raw):
    if "nc" not in _CACHE:
        _CACHE["nc"] = _build_nc()
    nc = _CACHE["nc"]
    in_maps = _prep_inputs(x, d_raw, st_h_raw, st_w_raw)
    res = run_bass_kernel_spmd(nc, in_maps, list(range(NCORES)))
    out = np.concatenate(
        [
            np.stack(
                [np.asarray(r[f"y{t}"]).reshape(H, W, C) for t in range(NTILES)]
            )
            for r in res.results
        ],
        axis=0,
    )
    return out
